# revision 1
# baseline (speedup 1.0000x reference)
"""Trainium2 Bass kernel for sliding-window causal self-attention (GQA + RoPE +
RMS-norm QK + value-embedding gating).

Sharding: 8 cores = 2 (batch) x 4 (KV groups).  Each core handles one batch
element and one KV head (= 4 query heads), computes a partial output through
the row-slice of Wproj for its heads; the host sums the 4 partials per batch.
"""

import sys
import os

for _p in ("/root/.axon_site", "/root/.axon_site/_ro/trn_rl_repo",
           "/root/.axon_site/_ro/pypackages", "/opt/trn_rl_repo"):
    if os.path.isdir(_p) and _p not in sys.path:
        sys.path.append(_p)

import numpy as np
import ml_dtypes
from contextlib import ExitStack

import concourse.bass as bass
import concourse.tile as tile
from concourse import bacc, mybir
from concourse.bass_utils import run_bass_kernel_spmd

BF16 = ml_dtypes.bfloat16
N_HEAD, N_KV, HEAD_DIM, WINDOW, N_EMBD = 16, 4, 64, 512, 1024
B, T = 2, 2048
NCORES = 8
TCH = 512               # token chunk for the projection phase
NCH = T // TCH          # 4
NTT = T // 128          # 16 t-tiles
HPK = N_HEAD // N_KV    # 4 query heads per core

F32 = mybir.dt.float32
BF = mybir.dt.bfloat16
AF = mybir.ActivationFunctionType
OP = mybir.AluOpType

_cache = {}


def _build(debug_taps=False):
    nc = bacc.Bacc("TRN2", target_bir_lowering=False, debug=False,
                   num_devices=NCORES)

    xt_d = nc.dram_tensor("xt", [8, 128, T], BF, kind="ExternalInput")
    wq_d = nc.dram_tensor("wq", [8, 128, 256], BF, kind="ExternalInput")
    wkv_d = nc.dram_tensor("wkv", [8, 128, 128], BF, kind="ExternalInput")
    wg_d = nc.dram_tensor("wg", [32, 1], BF, kind="ExternalInput")
    wp_d = nc.dram_tensor("wp", [2, 128, 1024], BF, kind="ExternalInput")
    cs1_d = nc.dram_tensor("cs1", [128, T], F32, kind="ExternalInput")
    cs2_d = nc.dram_tensor("cs2", [128, T], F32, kind="ExternalInput")
    ve_d = nc.dram_tensor("ve2", [16, 128, 64], BF, kind="ExternalInput")
    msk_d = nc.dram_tensor("masks", [128, 256], BF, kind="ExternalInput")
    id_d = nc.dram_tensor("ident", [64, 64], BF, kind="ExternalInput")
    selq_d = nc.dram_tensor("selq", [128, 33], BF, kind="ExternalInput")
    on64_d = nc.dram_tensor("ones64", [64, 1], BF, kind="ExternalInput")
    on1x_d = nc.dram_tensor("ones1x64", [1, 64], BF, kind="ExternalInput")
    id1_d = nc.dram_tensor("id1", [1, 1], BF, kind="ExternalInput")
    out_d = nc.dram_tensor("out", [T, N_EMBD], F32, kind="ExternalOutput")
    if debug_taps:
        qdbg_d = nc.dram_tensor("qdbg", [4, 64, T], BF, kind="ExternalOutput")
        kdbg_d = nc.dram_tensor("kdbg", [64, T], BF, kind="ExternalOutput")
        vdbg_d = nc.dram_tensor("vdbg", [128, NTT * 65], BF, kind="ExternalOutput")
        rkdbg_d = nc.dram_tensor("rkdbg", [128, NTT], F32, kind="ExternalOutput")
        ytdbg_d = nc.dram_tensor("ytdbg", [2, 128, T], BF, kind="ExternalOutput")

    with tile.TileContext(nc) as tc, ExitStack() as ctx:
        pers = ctx.enter_context(tc.tile_pool(name="pers", bufs=1))
        work = ctx.enter_context(tc.tile_pool(name="work", bufs=2))
        ptw = ctx.enter_context(tc.tile_pool(name="ptw", bufs=6))
        outw = ctx.enter_context(tc.tile_pool(name="outw", bufs=3))
        # PSUM pools (8 banks total):
        pb512 = ctx.enter_context(tc.tile_pool(name="pb512", bufs=2, space="PSUM"))
        pb128 = ctx.enter_context(tc.tile_pool(name="pb128", bufs=2, space="PSUM"))
        pbyx = ctx.enter_context(tc.tile_pool(name="pbyx", bufs=2, space="PSUM"))
        pbsm = ctx.enter_context(tc.tile_pool(name="pbsm", bufs=2, space="PSUM"))

        # ---- persistent SBUF loads ----
        xt_sb = pers.tile([128, 8 * T], BF, tag="xt")
        for kt in range(8):
            nc.sync.dma_start(xt_sb[:, kt * T:(kt + 1) * T], xt_d[kt])
        wq_sb = pers.tile([128, 8 * 256], BF, tag="wq")
        for kt in range(8):
            nc.sync.dma_start(wq_sb[:, kt * 256:(kt + 1) * 256], wq_d[kt])
        wkv_sb = pers.tile([128, 8 * 128], BF, tag="wkv")
        for kt in range(8):
            nc.sync.dma_start(wkv_sb[:, kt * 128:(kt + 1) * 128], wkv_d[kt])
        wg_sb = pers.tile([32, 1], BF, tag="wg")
        nc.sync.dma_start(wg_sb[:], wg_d[:])
        wp_sb = pers.tile([128, 2 * 1024], BF, tag="wp")
        for p in range(2):
            nc.sync.dma_start(wp_sb[:, p * 1024:(p + 1) * 1024], wp_d[p])
        cs1_sb = pers.tile([128, T], F32, tag="cs1")
        nc.sync.dma_start(cs1_sb[:], cs1_d[:])
        cs2_sb = pers.tile([128, T], F32, tag="cs2")
        nc.sync.dma_start(cs2_sb[:], cs2_d[:])
        ve_sb = pers.tile([128, 16 * 64], BF, tag="ve")
        for j in range(16):
            nc.sync.dma_start(ve_sb[:, j * 64:(j + 1) * 64], ve_d[j])
        msk_sb = pers.tile([128, 256], BF, tag="msk")
        nc.sync.dma_start(msk_sb[:], msk_d[:])
        id_sb = pers.tile([64, 64], BF, tag="ident")
        nc.sync.dma_start(id_sb[:], id_d[:])
        selq_sb = pers.tile([128, 33], BF, tag="selq")
        nc.sync.dma_start(selq_sb[:], selq_d[:])
        on64_sb = pers.tile([64, 1], BF, tag="on64")
        nc.sync.dma_start(on64_sb[:], on64_d[:])
        on1x_sb = pers.tile([1, 64], BF, tag="on1x")
        nc.sync.dma_start(on1x_sb[:], on1x_d[:])
        id1_sb = pers.tile([1, 1], BF, tag="id1")
        nc.sync.dma_start(id1_sb[:], id1_d[:])

        # ---- persistent intermediates ----
        qt_sb = [pers.tile([64, T], BF, tag=f"qt{h}", name=f"qt{h}")
                 for h in range(4)]          # Q^T per head
        kt_sb = pers.tile([64, T], BF, tag="kt")     # K^T
        vn_sb = pers.tile([128, NTT * 65], BF, tag="vn")  # V natural + ones col
        yt_sb = [pers.tile([128, T], BF, tag=f"yt{p}", name=f"yt{p}")
                 for p in range(2)]          # y^T, heads stacked
        rk_sb = pers.tile([128, NTT], F32, tag="rk")  # K rms recip, natural

        nc.vector.memset(vn_sb[:], 1.0)      # ones columns (col 64 of each group)
        biasq_sb = pers.tile([128, 1], F32, tag="biasq")
        nc.vector.memset(biasq_sb[:], 64e-6)
        biask_sb = pers.tile([1, 1], F32, tag="biask")
        nc.vector.memset(biask_sb[:], 1e-6)

        # =========== Phase 1: projections + RoPE + RMS + V prep ===========
        for ch in range(NCH):
            c0 = ch * TCH
            csl = slice(c0, c0 + TCH)

            def qk_head_ops(ps, rows, dst, is_q):
                """RoPE + RMS for psum rows [rows, rows+64*n) -> dst slice."""
                n = (128 - rows) // 64 if not is_q else 2
                # rope: A = ps * cs1, B = ps * cs2 (row-aligned trig tiles)
                nr = 128 if is_q else 64
                # A rows hh:    x1*cos   | B rows hh:    x2*sin (shifted up)
                # A rows hh+32: x1*sin(dn)| B rows hh+32: x2*cos
                A = work.tile([128, TCH], F32, tag="ropeA", name="ropeA")
                Bt = work.tile([128, TCH], F32, tag="ropeB", name="ropeB")
                ro = work.tile([128, TCH], F32, tag="rope", name="rope")
                for hh in range(0, nr, 64):
                    h1 = slice(hh, hh + 32)
                    h2 = slice(hh + 32, hh + 64)
                    nc.vector.tensor_mul(A[h1], ps[h1], cs1_sb[h1, csl])
                    nc.vector.tensor_mul(Bt[h1], ps[h2], cs1_sb[h2, csl])
                    nc.vector.tensor_mul(A[h2], ps[h1], cs2_sb[h1, csl])
                    nc.vector.tensor_mul(Bt[h2], ps[h2], cs2_sb[h2, csl])
                    nc.vector.tensor_sub(ro[h1], A[h1], Bt[h1])
                    nc.vector.tensor_add(ro[h2], A[h2], Bt[h2])
                # rms: recip = 1/sqrt(ss*sc + bias); Q folds the 1/8 score scale
                sq = work.tile([128, TCH], BF, tag="sq", name="sq")
                nc.scalar.square(sq[:nr], ro[0:nr])
                if is_q:
                    ss = pb512.tile([33, TCH], F32, tag="b512", name="ssq")
                    nc.tensor.matmul(ss[:], selq_sb[:], sq[:], start=True,
                                     stop=True)
                    bcps = pbyx.tile([128, TCH], F32, tag="yx", name="bcps")
                    for i in range(2):
                        r = 32 * i
                        srt = work.tile([1, TCH], F32, tag=f"srt{i}",
                                        name=f"srt{i}")
                        nc.scalar.activation(srt[:], ss[r:r + 1], AF.Sqrt,
                                             bias=biasq_sb[r:r + 1], scale=1.0)
                        rcpf = work.tile([1, TCH], F32, tag=f"rcpf{i}",
                                         name=f"rcpf{i}")
                        nc.vector.reciprocal_approx_fast(rcpf[:], srt[:])
                        rcp = work.tile([1, TCH], BF, tag=f"rcp{i}",
                                        name=f"rcp{i}")
                        nc.scalar.copy(rcp[:], rcpf[:])
                        nc.tensor.matmul(bcps[64 * i:64 * i + 64], on1x_sb[:],
                                         rcp[:], start=True, stop=True)
                    for i in range(2):
                        nc.vector.tensor_mul(dst[i][:, csl],
                                             ro[64 * i:64 * i + 64],
                                             bcps[64 * i:64 * i + 64])
                else:
                    # K^T stays unnormalized; rms recip folded into exp scale
                    nc.vector.tensor_copy(dst, ro[0:nr])
                    ss = pb512.tile([33, TCH], F32, tag="b512", name="ssk")
                    nc.tensor.matmul(ss[0:1], on64_sb[:], sq[0:64], start=True,
                                     stop=True)
                    srt = work.tile([1, TCH], F32, tag="srt0", name="srtk")
                    nc.scalar.activation(srt[:], ss[0:1], AF.Sqrt,
                                         bias=biask_sb[:], scale=1.0 / 64)
                    rcpkf = work.tile([1, TCH], F32, tag="rcpf0",
                                      name="rcpkf")
                    nc.vector.reciprocal_approx_fast(rcpkf[:], srt[:])
                    rcpk = work.tile([1, TCH], BF, tag="rcp0", name="rcpk")
                    nc.scalar.copy(rcpk[:], rcpkf[:])
                    for j in range(4):
                        rkp = pbsm.tile([128, 1], BF, tag="sm", name="rkp")
                        nc.tensor.transpose(
                            rkp[:], rcpk[:, j * 128:(j + 1) * 128], id1_sb[:])
                        tt = ch * 4 + j
                        nc.scalar.copy(rk_sb[:, tt:tt + 1], rkp[:])

            # Q pairs
            for p in range(2):
                psq = pb512.tile([128, TCH], F32, tag="b512", name="psq")
                for kt in range(8):
                    nc.tensor.matmul(
                        psq[:], wq_sb[:, kt * 256 + p * 128: kt * 256 + (p + 1) * 128],
                        xt_sb[:, kt * T + c0: kt * T + c0 + TCH],
                        start=(kt == 0), stop=(kt == 7))
                qk_head_ops(psq, 0, qt_sb[2 * p:2 * p + 2], True)

            # K | V^T
            pskv = pb512.tile([128, TCH], F32, tag="b512", name="pskv")
            for kt in range(8):
                nc.tensor.matmul(
                    pskv[:], wkv_sb[:, kt * 128:(kt + 1) * 128],
                    xt_sb[:, kt * T + c0: kt * T + c0 + TCH],
                    start=(kt == 0), stop=(kt == 7))
            qk_head_ops(pskv, 0, kt_sb[:, csl], False)
            vt_bf = work.tile([64, TCH], BF, tag="vt", name="vt")
            nc.scalar.copy(vt_bf[:], pskv[64:128])

            # V natural (+ gate * ve) per t-tile
            for j in range(4):
                t0 = c0 + j * 128
                tt = ch * 4 + j
                vtp = pbsm.tile([128, 64], BF, tag="sm", name="vtp")
                nc.tensor.transpose(vtp[:], vt_bf[:, j * 128:(j + 1) * 128],
                                    id_sb[:])
                gps = pbsm.tile([128, 64], F32, tag="sm", name="gps")
                nc.tensor.matmul(gps[:, 0:1], xt_sb[0:32, t0:t0 + 128],
                                 wg_sb[:], start=True, stop=True)
                g_sb = outw.tile([128, 1], F32, tag="g", name="g")
                nc.scalar.activation(g_sb[:], gps[:, 0:1], AF.Sigmoid)
                nc.vector.scalar_tensor_tensor(
                    vn_sb[:, tt * 65: tt * 65 + 64],
                    ve_sb[:, tt * 64:(tt + 1) * 64], g_sb[:], vtp[:],
                    op0=OP.mult, op1=OP.add)

        # ====== Phase 2+3: attention per q-tile, then output projection ======
        for qt in range(NTT):
            lo = max(0, qt - 4)
            for h in range(HPK):
                p, hh = h // 2, (h % 2) * 64
                q_ap = qt_sb[h][:, qt * 128:(qt + 1) * 128]
                yext = pbyx.tile([65, 128], F32, tag="yx", name="yext")
                for kt in range(lo, qt + 1):
                    stp = pb128.tile([128, 128], F32, tag="st", name="stp")
                    nc.tensor.matmul(stp[:],
                                     kt_sb[:, kt * 128:(kt + 1) * 128], q_ap,
                                     start=True, stop=True)
                    pt = ptw.tile([128, 128], BF, tag="pt", name="pt")
                    nc.scalar.activation(pt[:], stp[:], AF.Exp,
                                         scale=rk_sb[:, kt:kt + 1])
                    if kt == qt:
                        nc.vector.tensor_mul(pt[:], pt[:], msk_sb[:, 0:128])
                    elif kt == qt - 4:
                        nc.vector.tensor_mul(pt[:], pt[:], msk_sb[:, 128:256])
                    nc.tensor.matmul(yext[:],
                                     vn_sb[:, kt * 65: kt * 65 + 65], pt[:],
                                     start=(kt == lo), stop=(kt == qt))
                rrf = outw.tile([1, 128], F32, tag="rrf", name="rrf")
                nc.vector.reciprocal(rrf[:], yext[64:65, :])
                rr = outw.tile([1, 128], BF, tag="rr", name="rr")
                nc.scalar.copy(rr[:], rrf[:])
                bcq = pbsm.tile([64, 128], F32, tag="sm", name="bcq")
                nc.tensor.matmul(bcq[:], on1x_sb[:], rr[:], start=True,
                                 stop=True)
                bca = outw.tile([64, 128], BF, tag="bca", name="bca")
                nc.scalar.copy(bca[:], bcq[:])
                nc.vector.tensor_mul(
                    yt_sb[p][hh:hh + 64, qt * 128:(qt + 1) * 128],
                    yext[0:64, :], bca[:])

            if debug_taps and qt == NTT - 1:
                for h in range(4):
                    nc.sync.dma_start(qdbg_d[h], qt_sb[h][:])
                nc.sync.dma_start(kdbg_d[:], kt_sb[:])
                nc.sync.dma_start(vdbg_d[:], vn_sb[:])
                nc.sync.dma_start(rkdbg_d[:], rk_sb[:])
            if debug_taps and qt == NTT - 1:
                for p in range(2):
                    nc.sync.dma_start(ytdbg_d[p], yt_sb[p][:])
            # output projection for this t-tile
            for cc in range(2):
                ops = pb512.tile([128, TCH], F32, tag="b512", name="ops")
                for p in range(2):
                    nc.tensor.matmul(
                        ops[:], yt_sb[p][:, qt * 128:(qt + 1) * 128],
                        wp_sb[:, p * 1024 + cc * 512: p * 1024 + cc * 512 + 512],
                        start=(p == 0), stop=(p == 1))
                o_sb = outw.tile([128, TCH], F32, tag="osb", name="osb")
                if cc == 0:
                    nc.scalar.copy(o_sb[:], ops[:])
                else:
                    nc.vector.tensor_copy(o_sb[:], ops[:])
                nc.sync.dma_start(
                    out_d[qt * 128:(qt + 1) * 128, cc * 512:(cc + 1) * 512],
                    o_sb[:])

    nc.compile()
    return nc


def _prep_inputs(x, ve, cos, sin, Wq, Wk, Wv, Wproj, Wgate):
    """Build the 8 per-core input maps (host-side sharding + layout prep)."""
    cosT = np.ascontiguousarray(cos.T).astype(np.float32)   # [32, T]
    sinT = np.ascontiguousarray(sin.T).astype(np.float32)
    cs1 = np.concatenate([cosT, sinT, cosT, sinT], 0)       # [128, T]
    cs2 = np.concatenate([sinT, cosT, sinT, cosT], 0)
    masks = np.concatenate([
        np.triu(np.ones((128, 128), np.float32)),           # causal (col>=row)
        np.tril(np.ones((128, 128), np.float32)),           # window (col<=row)
    ], 1).astype(BF16)
    ident = np.eye(64, dtype=BF16)
    selq = np.zeros((128, 33), np.float32)
    selq[0:64, 0] = 1.0
    selq[64:128, 32] = 1.0
    selq = selq.astype(BF16)
    ones64 = np.ones((64, 1), BF16)
    ones1x64 = np.ones((1, 64), BF16)
    id1 = np.ones((1, 1), BF16)

    xT = [np.ascontiguousarray(x[b].astype(BF16).T).reshape(8, 128, T)
          for b in range(B)]
    in_maps = []
    for c in range(NCORES):
        b, g = c // 4, c % 4
        wq_g = np.ascontiguousarray(
            Wq[:, g * 256:(g + 1) * 256]).astype(BF16).reshape(8, 128, 256)
        wkv_g = np.concatenate(
            [Wk[:, g * 64:(g + 1) * 64], Wv[:, g * 64:(g + 1) * 64]],
            1).astype(BF16).reshape(8, 128, 128)
        wg_g = np.ascontiguousarray(Wgate[:, g:g + 1]).astype(BF16)
        wp_g = np.ascontiguousarray(
            Wproj[g * 256:(g + 1) * 256, :]).astype(BF16).reshape(2, 128, 1024)
        ve_g = np.ascontiguousarray(
            2.0 * ve[b, :, g * 64:(g + 1) * 64]).astype(BF16).reshape(16, 128, 64)
        in_maps.append({
            "xt": xT[b], "wq": wq_g, "wkv": wkv_g, "wg": wg_g, "wp": wp_g,
            "cs1": cs1, "cs2": cs2, "ve2": ve_g, "masks": masks,
            "ident": ident, "selq": selq, "ones64": ones64,
            "ones1x64": ones1x64, "id1": id1,
        })
    return in_maps


def _run(inputs, trace=False, tmpdir=None):
    if "nc" not in _cache:
        _cache["nc"] = _build()
    nc = _cache["nc"]
    in_maps = _prep_inputs(**inputs)
    res = run_bass_kernel_spmd(nc, in_maps, list(range(NCORES)), trace=trace,
                               tmpdir=tmpdir)
    out = np.zeros((B, T, N_EMBD), np.float32)
    for c in range(NCORES):
        out[c // 4] += np.asarray(res.results[c]["out"], np.float32)
    return out, res


def kernel(**inputs):
    out, _ = _run(inputs)
    return out



# revision 7
# speedup vs baseline: 1.1838x; 1.1838x over previous
"""Trainium2 Bass kernel for sliding-window causal self-attention (GQA + RoPE +
RMS-norm QK + value-embedding gating).

Sharding: 8 cores = 2 (batch) x 4 (KV groups).  Each core handles one batch
element and one KV head (= 4 query heads), computes a partial output through
the row-slice of Wproj for its heads; the host sums the 4 partials per batch.

v2: 4-head-batched attention (FD=512 ops), RMS recip via Ln/Exp (single
activation table set), gate sigmoid via Exp+reciprocal, RoPE in bf16 SBUF,
Q normalization folded before RoPE (linearity), bf16 output.
"""

import sys
import os

for _p in ("/root/.axon_site", "/root/.axon_site/_ro/trn_rl_repo",
           "/root/.axon_site/_ro/pypackages", "/opt/trn_rl_repo"):
    if os.path.isdir(_p) and _p not in sys.path:
        sys.path.append(_p)

import numpy as np
import ml_dtypes
from contextlib import ExitStack

import concourse.bass as bass
import concourse.tile as tile
from concourse import bacc, mybir
from concourse.bass_utils import run_bass_kernel_spmd

BF16 = ml_dtypes.bfloat16
N_HEAD, N_KV, HEAD_DIM, WINDOW, N_EMBD = 16, 4, 64, 512, 1024
B, T = 2, 2048
NCORES = 8
TCH = 512               # token chunk for the projection phase
NCH = T // TCH          # 4
NTT = T // 128          # 16 t-tiles

F32 = mybir.dt.float32
BF = mybir.dt.bfloat16
AF = mybir.ActivationFunctionType
OP = mybir.AluOpType

_cache = {}


def _build():
    nc = bacc.Bacc("TRN2", target_bir_lowering=False, debug=False,
                   num_devices=NCORES)

    xt_d = nc.dram_tensor("xt", [8, 128, T], BF, kind="ExternalInput")
    wq_d = nc.dram_tensor("wq", [8, 128, 256], BF, kind="ExternalInput")
    wkv_d = nc.dram_tensor("wkv", [8, 128, 128], BF, kind="ExternalInput")
    wg_d = nc.dram_tensor("wg", [32, 1], BF, kind="ExternalInput")
    wp_d = nc.dram_tensor("wp", [2, 128, 1024], BF, kind="ExternalInput")
    cs1_d = nc.dram_tensor("cs1", [128, T], BF, kind="ExternalInput")
    cs2_d = nc.dram_tensor("cs2", [128, T], BF, kind="ExternalInput")
    ve_d = nc.dram_tensor("ve2", [16, 128, 64], BF, kind="ExternalInput")
    msk_d = nc.dram_tensor("masks", [128, 1024], BF, kind="ExternalInput")
    id_d = nc.dram_tensor("ident", [64, 64], BF, kind="ExternalInput")
    selq_d = nc.dram_tensor("selq2", [128, 2], BF, kind="ExternalInput")
    sel128_d = nc.dram_tensor("sel128", [2, 128], BF, kind="ExternalInput")
    on64_d = nc.dram_tensor("ones64", [64, 1], BF, kind="ExternalInput")
    on1x_d = nc.dram_tensor("ones1x64", [1, 64], BF, kind="ExternalInput")
    id1_d = nc.dram_tensor("id1", [1, 1], BF, kind="ExternalInput")
    out_d = nc.dram_tensor("out", [T, N_EMBD], BF, kind="ExternalOutput")

    with tile.TileContext(nc) as tc, ExitStack() as ctx:
        pers = ctx.enter_context(tc.tile_pool(name="pers", bufs=1))
        wk = ctx.enter_context(tc.tile_pool(name="wk", bufs=4))
        ptp = ctx.enter_context(tc.tile_pool(name="ptp", bufs=4))
        sm = ctx.enter_context(tc.tile_pool(name="sm", bufs=4))
        ow = ctx.enter_context(tc.tile_pool(name="ow", bufs=3))
        pbA = ctx.enter_context(tc.tile_pool(name="pbA", bufs=4, space="PSUM"))
        pbY = ctx.enter_context(tc.tile_pool(name="pbY", bufs=2, space="PSUM"))
        pbS = ctx.enter_context(tc.tile_pool(name="pbS", bufs=2, space="PSUM"))

        # ---- persistent SBUF loads ----
        xt_sb = []
        for k in range(8):
            t_ = pers.tile([128, T], BF, tag=f"xt{k}", name=f"xt{k}")
            nc.sync.dma_start(t_[:], xt_d[k])
            xt_sb.append(t_)
        cs1_sb = pers.tile([128, T], BF, tag="cs1")
        nc.sync.dma_start(cs1_sb[:], cs1_d[:])
        cs2_sb = pers.tile([128, T], BF, tag="cs2")
        nc.sync.dma_start(cs2_sb[:], cs2_d[:])
        wq_sb = pers.tile([128, 8 * 256], BF, tag="wq")
        for k in range(8):
            nc.sync.dma_start(wq_sb[:, k * 256:(k + 1) * 256], wq_d[k])
        wkv_sb = pers.tile([128, 8 * 128], BF, tag="wkv")
        for k in range(8):
            nc.sync.dma_start(wkv_sb[:, k * 128:(k + 1) * 128], wkv_d[k])
        wg_sb = pers.tile([32, 1], BF, tag="wg")
        nc.sync.dma_start(wg_sb[:], wg_d[:])
        ve_sb = pers.tile([128, 16 * 64], BF, tag="ve")
        for j in range(16):
            nc.sync.dma_start(ve_sb[:, j * 64:(j + 1) * 64], ve_d[j])
        mskc_sb = pers.tile([128, 512], BF, tag="mskc")
        nc.sync.dma_start(mskc_sb[:], msk_d[:, 0:512])
        mskw_sb = pers.tile([128, 512], BF, tag="mskw")
        nc.sync.dma_start(mskw_sb[:], msk_d[:, 512:1024])
        id_sb = pers.tile([64, 64], BF, tag="ident")
        nc.sync.dma_start(id_sb[:], id_d[:])
        selq_sb = pers.tile([128, 2], BF, tag="selq2")
        nc.sync.dma_start(selq_sb[:], selq_d[:])
        sel128_sb = pers.tile([2, 128], BF, tag="sel128")
        nc.sync.dma_start(sel128_sb[:], sel128_d[:])
        on64_sb = pers.tile([64, 1], BF, tag="on64")
        nc.sync.dma_start(on64_sb[:], on64_d[:])
        on1x_sb = pers.tile([1, 64], BF, tag="on1x")
        nc.sync.dma_start(on1x_sb[:], on1x_d[:])
        id1_sb = pers.tile([1, 1], BF, tag="id1")
        nc.sync.dma_start(id1_sb[:], id1_d[:])
        wp_sb = pers.tile([128, 2 * 1024], BF, tag="wp")
        for p in range(2):
            nc.sync.dma_start(wp_sb[:, p * 1024:(p + 1) * 1024], wp_d[p])

        # ---- persistent intermediates ----
        # Q^T, 4 heads side-by-side per q-tile: [64, qt(16) x h(4) x 128]
        q4t = pers.tile([64, NTT * 512], BF, tag="q4t")
        kt_sb = pers.tile([64, T], BF, tag="kt")      # K^T (un-normalized)
        vn_sb = pers.tile([128, NTT * 65], BF, tag="vn")  # V natural + ones col
        # y^T: [128 (2 heads stacked), p(2) x T]
        yt_sb = pers.tile([128, 2 * T], BF, tag="yt")
        rk_sb = pers.tile([128, NTT], F32, tag="rk")  # K rms recip, natural
        g_sb = pers.tile([128, NTT], F32, tag="g")    # sigmoid gates, natural

        nc.vector.memset(vn_sb[:], 1.0)      # ones columns (col 64 of each group)
        biasq_sb = pers.tile([2, 1], F32, tag="biasq")
        nc.vector.memset(biasq_sb[:], 64e-6)
        biask_sb = pers.tile([1, 1], F32, tag="biask")
        nc.vector.memset(biask_sb[:], 1e-6)

        # ---- gates for all t-tiles (sigmoid via exp + reciprocal) ----
        gps = pbS.tile([128, NTT], F32, tag="s", name="gps")
        for tt in range(NTT):
            nc.tensor.matmul(gps[:, tt:tt + 1],
                             xt_sb[0][0:32, tt * 128:(tt + 1) * 128],
                             wg_sb[:], start=True, stop=True)
        eg = sm.tile([128, NTT], F32, tag="u", name="eg")
        nc.scalar.activation(eg[:], gps[:], AF.Exp, scale=-1.0)
        eg1 = sm.tile([128, NTT], F32, tag="u", name="eg1")
        nc.vector.tensor_scalar_add(eg1[:], eg[:], 1.0)
        nc.vector.reciprocal(g_sb[:], eg1[:])

        def qkv_matmul(psum, w_sb, col0, ncol, c0):
            for k in range(8):
                nc.tensor.matmul(
                    psum, w_sb[:, k * ncol + col0: k * ncol + col0 + 128],
                    xt_sb[k][:, c0:c0 + TCH],
                    start=(k == 0), stop=(k == 7))

        for ch in range(NCH):
            c0 = ch * TCH
            csl = slice(c0, c0 + TCH)
            # view of this chunk's q4t region: [64, j(4 qtiles), h(4), c(128)]
            q4v = q4t[:, ch * 2048:(ch + 1) * 2048].rearrange(
                "p (j h c) -> p j h c", j=4, h=4, c=128)

            # ---------------- Q pairs ----------------
            for p in range(2):
                psq = pbA.tile([128, TCH], F32, tag="A", name="psq")
                qkv_matmul(psq, wq_sb, p * 128, 256, c0)
                sq = wk.tile([128, TCH], BF, tag="w", name="sq")
                nc.scalar.square(sq[:], psq[:])
                ss = pbS.tile([2, TCH], F32, tag="s", name="ss")
                nc.tensor.matmul(ss[:], selq_sb[:], sq[:], start=True,
                                 stop=True)
                u = sm.tile([2, TCH], F32, tag="u", name="uq")
                nc.scalar.activation(u[:], ss[:], AF.Ln, bias=biasq_sb[:],
                                     scale=1.0)
                rcpb = sm.tile([2, TCH], BF, tag="rc", name="rcpb")
                nc.scalar.activation(rcpb[:], u[:], AF.Exp, scale=-0.5)
                bcps = pbA.tile([128, TCH], F32, tag="A", name="bcps")
                nc.tensor.matmul(bcps[:], sel128_sb[:], rcpb[:], start=True,
                                 stop=True)
                bcs = wk.tile([128, TCH], BF, tag="w", name="bcs")
                nc.vector.tensor_copy(bcs[:], bcps[:])
                pbn = wk.tile([128, TCH], BF, tag="w", name="pbn")
                nc.vector.tensor_mul(pbn[:], psq[:], bcs[:])
                # A rows: [x1*cos | x2*cos'...]: A = pbn * cs1 -> [x1c, x2s]
                # P2 = pbn * cs2 -> [x1s, x2c]
                # Bsh[h1] = x2*sin (from pbn[h2]*cs1[h2], inputs aligned)
                # Bsh[h2] = x1*sin (from pbn[h1]*cs2[h1], inputs aligned)
                A = wk.tile([128, TCH], BF, tag="w", name="ropeA")
                P2 = wk.tile([128, TCH], BF, tag="w", name="ropeP2")
                Bs = wk.tile([128, TCH], BF, tag="w", name="ropeBs")
                nc.vector.tensor_mul(A[:], pbn[:], cs1_sb[:, csl])
                nc.vector.tensor_mul(P2[:], pbn[:], cs2_sb[:, csl])
                for i in range(2):
                    h = 2 * p + i
                    hh = 64 * i
                    h1 = slice(hh, hh + 32)
                    h2 = slice(hh + 32, hh + 64)
                    nc.vector.tensor_mul(Bs[h1], pbn[h2], cs1_sb[h2, csl])
                    nc.vector.tensor_mul(Bs[h2], pbn[h1], cs2_sb[h1, csl])
                    # out[:32] = x1c - x2s ; out[32:64] = x1s + x2c
                    nc.vector.tensor_sub(q4v[0:32, :, h, :], A[h1], Bs[h1])
                    nc.vector.tensor_add(q4v[32:64, :, h, :], Bs[h2], P2[h2])

            # ---------------- K | V^T ----------------
            pskv = pbA.tile([128, TCH], F32, tag="A", name="pskv")
            qkv_matmul(pskv, wkv_sb, 0, 128, c0)
            sqk = wk.tile([64, TCH], BF, tag="w", name="sqk")
            nc.scalar.square(sqk[:], pskv[0:64])
            ssk = pbS.tile([1, TCH], F32, tag="s", name="ssk")
            nc.tensor.matmul(ssk[:], on64_sb[:], sqk[:], start=True, stop=True)
            uk = sm.tile([1, TCH], F32, tag="u", name="uk")
            nc.scalar.activation(uk[:], ssk[:], AF.Ln, bias=biask_sb[:],
                                 scale=1.0 / 64)
            rkb = sm.tile([1, TCH], BF, tag="rc", name="rkb")
            nc.scalar.activation(rkb[:], uk[:], AF.Exp, scale=-0.5)
            for j in range(4):
                tt = ch * 4 + j
                rkp = pbS.tile([128, 1], BF, tag="s", name="rkp")
                nc.tensor.transpose(rkp[:], rkb[:, j * 128:(j + 1) * 128],
                                    id1_sb[:])
                nc.scalar.copy(rk_sb[:, tt:tt + 1], rkp[:])
            pbk = wk.tile([64, TCH], BF, tag="w", name="pbk")
            nc.vector.tensor_copy(pbk[:], pskv[0:64])
            Ak = wk.tile([64, TCH], BF, tag="w", name="ropeAk")
            Pk = wk.tile([64, TCH], BF, tag="w", name="ropePk")
            Bsk = wk.tile([64, TCH], BF, tag="w", name="ropeBsk")
            nc.vector.tensor_mul(Ak[:], pbk[:], cs1_sb[0:64, csl])
            nc.vector.tensor_mul(Pk[:], pbk[:], cs2_sb[0:64, csl])
            nc.vector.tensor_mul(Bsk[0:32], pbk[32:64], cs1_sb[32:64, csl])
            nc.vector.tensor_mul(Bsk[32:64], pbk[0:32], cs2_sb[0:32, csl])
            nc.vector.tensor_sub(kt_sb[0:32, csl], Ak[0:32], Bsk[0:32])
            nc.vector.tensor_add(kt_sb[32:64, csl], Bsk[32:64], Pk[32:64])
            # V natural (+ gate * ve) per t-tile
            vt = wk.tile([64, TCH], BF, tag="w", name="vt")
            nc.scalar.copy(vt[:], pskv[64:128])
            for j in range(4):
                tt = ch * 4 + j
                vtp = pbS.tile([128, 64], BF, tag="s", name="vtp")
                nc.tensor.transpose(vtp[:], vt[:, j * 128:(j + 1) * 128],
                                    id_sb[:])
                nc.vector.scalar_tensor_tensor(
                    vn_sb[:, tt * 65: tt * 65 + 64],
                    ve_sb[:, tt * 64:(tt + 1) * 64], g_sb[:, tt:tt + 1],
                    vtp[:], op0=OP.mult, op1=OP.add)

            # ============ attention + projection for this chunk ============
            for qt in range(ch * 4, ch * 4 + 4):
                lo = max(0, qt - 4)
                q_ap = q4t[:, qt * 512:(qt + 1) * 512]
                yext = pbY.tile([65, TCH], F32, tag="y", name="yext")
                for kt in range(lo, qt + 1):
                    st = pbA.tile([128, TCH], F32, tag="A", name="st")
                    nc.tensor.matmul(st[:],
                                     kt_sb[:, kt * 128:(kt + 1) * 128], q_ap,
                                     start=True, stop=True)
                    pt = ptp.tile([128, TCH], BF, tag="pt", name="pt")
                    nc.scalar.activation(pt[:], st[:], AF.Exp,
                                         scale=rk_sb[:, kt:kt + 1])
                    if kt == qt:
                        nc.vector.tensor_mul(pt[:], pt[:], mskc_sb[:])
                    elif kt == qt - 4:
                        nc.vector.tensor_mul(pt[:], pt[:], mskw_sb[:])
                    nc.tensor.matmul(yext[:],
                                     vn_sb[:, kt * 65: kt * 65 + 65], pt[:],
                                     start=(kt == lo), stop=(kt == qt))
                u2 = sm.tile([1, TCH], F32, tag="u", name="u2")
                nc.scalar.activation(u2[:], yext[64:65, :], AF.Ln)
                rrb = sm.tile([1, TCH], BF, tag="rc", name="rrb")
                nc.scalar.activation(rrb[:], u2[:], AF.Exp, scale=-1.0)
                bcq = pbA.tile([64, TCH], F32, tag="A", name="bcq")
                nc.tensor.matmul(bcq[:], on1x_sb[:], rrb[:], start=True,
                                 stop=True)
                bca = ow.tile([64, TCH], BF, tag="bca", name="bca")
                nc.vector.tensor_copy(bca[:], bcq[:])
                for h in range(4):
                    p, hh = h // 2, (h % 2) * 64
                    nc.vector.tensor_mul(
                        yt_sb[hh:hh + 64, p * T + qt * 128: p * T + (qt + 1) * 128],
                        yext[0:64, h * 128:(h + 1) * 128],
                        bca[:, h * 128:(h + 1) * 128])
                # output projection for this t-tile
                for cc in range(2):
                    ops = pbA.tile([128, TCH], F32, tag="A", name="ops")
                    for p in range(2):
                        nc.tensor.matmul(
                            ops[:], yt_sb[:, p * T + qt * 128: p * T + (qt + 1) * 128],
                            wp_sb[:, p * 1024 + cc * 512: p * 1024 + cc * 512 + 512],
                            start=(p == 0), stop=(p == 1))
                    o_sb = ow.tile([128, TCH], BF, tag="o", name="osb")
                    if cc == 0:
                        nc.scalar.copy(o_sb[:], ops[:])
                    else:
                        nc.vector.tensor_copy(o_sb[:], ops[:])
                    nc.sync.dma_start(
                        out_d[qt * 128:(qt + 1) * 128, cc * 512:(cc + 1) * 512],
                        o_sb[:])

    nc.compile()
    return nc


def _prep_inputs(x, ve, cos, sin, Wq, Wk, Wv, Wproj, Wgate):
    """Build the 8 per-core input maps (host-side sharding + layout prep)."""
    cosT = np.ascontiguousarray(cos.T).astype(np.float32)   # [32, T]
    sinT = np.ascontiguousarray(sin.T).astype(np.float32)
    cs1 = np.concatenate([cosT, sinT, cosT, sinT], 0).astype(BF16)  # [128, T]
    cs2 = np.concatenate([sinT, cosT, sinT, cosT], 0).astype(BF16)
    triu = np.triu(np.ones((128, 128), np.float32))
    tril = np.tril(np.ones((128, 128), np.float32))
    masks = np.concatenate([np.tile(triu, (1, 4)), np.tile(tril, (1, 4))],
                           1).astype(BF16)                  # [128, 1024]
    ident = np.eye(64, dtype=BF16)
    selq2 = np.zeros((128, 2), np.float32)
    selq2[0:64, 0] = 1.0
    selq2[64:128, 1] = 1.0
    selq2 = selq2.astype(BF16)
    sel128 = np.zeros((2, 128), np.float32)
    sel128[0, 0:64] = 1.0
    sel128[1, 64:128] = 1.0
    sel128 = sel128.astype(BF16)
    ones64 = np.ones((64, 1), BF16)
    ones1x64 = np.ones((1, 64), BF16)
    id1 = np.ones((1, 1), BF16)

    xT = [np.ascontiguousarray(x[b].astype(BF16).T).reshape(8, 128, T)
          for b in range(B)]
    in_maps = []
    for c in range(NCORES):
        b, g = c // 4, c % 4
        wq_g = np.ascontiguousarray(
            Wq[:, g * 256:(g + 1) * 256]).astype(BF16).reshape(8, 128, 256)
        wkv_g = np.concatenate(
            [Wk[:, g * 64:(g + 1) * 64], Wv[:, g * 64:(g + 1) * 64]],
            1).astype(BF16).reshape(8, 128, 128)
        wg_g = np.ascontiguousarray(Wgate[:, g:g + 1]).astype(BF16)
        wp_g = np.ascontiguousarray(
            Wproj[g * 256:(g + 1) * 256, :]).astype(BF16).reshape(2, 128, 1024)
        ve_g = np.ascontiguousarray(
            2.0 * ve[b, :, g * 64:(g + 1) * 64]).astype(BF16).reshape(16, 128, 64)
        in_maps.append({
            "xt": xT[b], "wq": wq_g, "wkv": wkv_g, "wg": wg_g, "wp": wp_g,
            "cs1": cs1, "cs2": cs2, "ve2": ve_g, "masks": masks,
            "ident": ident, "selq2": selq2, "sel128": sel128,
            "ones64": ones64, "ones1x64": ones1x64, "id1": id1,
        })
    return in_maps


def _run(inputs, trace=False, tmpdir=None):
    if "nc" not in _cache:
        _cache["nc"] = _build()
    nc = _cache["nc"]
    in_maps = _prep_inputs(**inputs)
    res = run_bass_kernel_spmd(nc, in_maps, list(range(NCORES)), trace=trace,
                               tmpdir=tmpdir)
    out = np.zeros((B, T, N_EMBD), np.float32)
    for c in range(NCORES):
        out[c // 4] += np.asarray(res.results[c]["out"]).astype(np.float32)
    return out, res


def kernel(**inputs):
    out, _ = _run(inputs)
    return out


# revision 11
# speedup vs baseline: 1.4276x; 1.2060x over previous
"""Trainium2 Bass kernel for sliding-window causal self-attention (GQA + RoPE +
RMS-norm QK + value-embedding gating).

Sharding: 8 cores = 2 (batch) x 4 (KV groups).  Each core handles one batch
element and one KV head (= 4 query heads), computes a partial output through
the row-slice of Wproj for its heads; the host sums the 4 partials per batch.

v2: 4-head-batched attention (FD=512 ops), RMS recip via Ln/Exp (single
activation table set), gate sigmoid via Exp+reciprocal, RoPE in bf16 SBUF,
Q normalization folded before RoPE (linearity), bf16 output.
"""

import sys
import os

for _p in ("/root/.axon_site", "/root/.axon_site/_ro/trn_rl_repo",
           "/root/.axon_site/_ro/pypackages", "/opt/trn_rl_repo"):
    if os.path.isdir(_p) and _p not in sys.path:
        sys.path.append(_p)

import numpy as np
import ml_dtypes
from contextlib import ExitStack

import concourse.bass as bass
import concourse.tile as tile
from concourse import bacc, mybir
from concourse.bass_utils import run_bass_kernel_spmd

BF16 = ml_dtypes.bfloat16
N_HEAD, N_KV, HEAD_DIM, WINDOW, N_EMBD = 16, 4, 64, 512, 1024
B, T = 2, 2048
NCORES = 8
TCH = 512               # token chunk for the projection phase
NCH = T // TCH          # 4
NTT = T // 128          # 16 t-tiles

F32 = mybir.dt.float32
BF = mybir.dt.bfloat16
AF = mybir.ActivationFunctionType
OP = mybir.AluOpType

_cache = {}


def _build():
    nc = bacc.Bacc("TRN2", target_bir_lowering=False, debug=False,
                   num_devices=NCORES)

    xt_d = nc.dram_tensor("xt", [8, 128, T], BF, kind="ExternalInput")
    wq_d = nc.dram_tensor("wq", [128, 8 * 256], BF, kind="ExternalInput")
    wkv_d = nc.dram_tensor("wkv", [128, 8 * 128], BF, kind="ExternalInput")
    wg_d = nc.dram_tensor("wg", [32, 1], BF, kind="ExternalInput")
    wp_d = nc.dram_tensor("wp", [128, 2 * 1024], BF, kind="ExternalInput")
    cs1_d = nc.dram_tensor("cs1", [128, T], BF, kind="ExternalInput")
    cs2_d = nc.dram_tensor("cs2", [128, T], BF, kind="ExternalInput")
    ve_d = nc.dram_tensor("ve2", [128, 16 * 64], BF, kind="ExternalInput")
    msk_d = nc.dram_tensor("masks", [128, 1024], BF, kind="ExternalInput")
    id_d = nc.dram_tensor("ident", [64, 64], BF, kind="ExternalInput")
    selq_d = nc.dram_tensor("selq2", [128, 2], BF, kind="ExternalInput")
    sel128_d = nc.dram_tensor("sel128", [2, 128], BF, kind="ExternalInput")
    on64_d = nc.dram_tensor("ones64", [64, 1], BF, kind="ExternalInput")
    on1x_d = nc.dram_tensor("ones1x64", [1, 64], BF, kind="ExternalInput")
    id1_d = nc.dram_tensor("id1", [1, 1], BF, kind="ExternalInput")
    out_d = nc.dram_tensor("out", [T, N_EMBD], BF, kind="ExternalOutput")

    with tile.TileContext(nc) as tc, ExitStack() as ctx:
        pers = ctx.enter_context(tc.tile_pool(name="pers", bufs=1))
        wk = ctx.enter_context(tc.tile_pool(name="wk", bufs=4))
        ptp = ctx.enter_context(tc.tile_pool(name="ptp", bufs=4))
        sm = ctx.enter_context(tc.tile_pool(name="sm", bufs=4))
        ow = ctx.enter_context(tc.tile_pool(name="ow", bufs=3))
        pbA = ctx.enter_context(tc.tile_pool(name="pbA", bufs=4, space="PSUM"))
        pbY = ctx.enter_context(tc.tile_pool(name="pbY", bufs=2, space="PSUM"))
        pbS = ctx.enter_context(tc.tile_pool(name="pbS", bufs=2, space="PSUM"))

        # ---- persistent SBUF loads ----
        xt_sb = []
        for k in range(8):
            t_ = pers.tile([128, T], BF, tag=f"xt{k}", name=f"xt{k}")
            nc.sync.dma_start(t_[:], xt_d[k])
            xt_sb.append(t_)
        cs1_sb = pers.tile([128, T], BF, tag="cs1")
        nc.sync.dma_start(cs1_sb[:], cs1_d[:])
        cs2_sb = pers.tile([128, T], BF, tag="cs2")
        nc.sync.dma_start(cs2_sb[:], cs2_d[:])
        wq_sb = pers.tile([128, 8 * 256], BF, tag="wq")
        nc.sync.dma_start(wq_sb[:], wq_d[:])
        wkv_sb = pers.tile([128, 8 * 128], BF, tag="wkv")
        nc.sync.dma_start(wkv_sb[:], wkv_d[:])
        wg_sb = pers.tile([32, 1], BF, tag="wg")
        nc.sync.dma_start(wg_sb[:], wg_d[:])
        ve_sb = pers.tile([128, 16 * 64], BF, tag="ve")
        nc.sync.dma_start(ve_sb[:], ve_d[:])
        mskc_sb = pers.tile([128, 512], BF, tag="mskc")
        nc.sync.dma_start(mskc_sb[:], msk_d[:, 0:512])
        mskw_sb = pers.tile([128, 512], BF, tag="mskw")
        nc.sync.dma_start(mskw_sb[:], msk_d[:, 512:1024])
        id_sb = pers.tile([64, 64], BF, tag="ident")
        nc.sync.dma_start(id_sb[:], id_d[:])
        selq_sb = pers.tile([128, 2], BF, tag="selq2")
        nc.sync.dma_start(selq_sb[:], selq_d[:])
        sel128_sb = pers.tile([2, 128], BF, tag="sel128")
        nc.sync.dma_start(sel128_sb[:], sel128_d[:])
        on64_sb = pers.tile([64, 1], BF, tag="on64")
        nc.sync.dma_start(on64_sb[:], on64_d[:])
        on1x_sb = pers.tile([1, 64], BF, tag="on1x")
        nc.sync.dma_start(on1x_sb[:], on1x_d[:])
        id1_sb = pers.tile([1, 1], BF, tag="id1")
        nc.sync.dma_start(id1_sb[:], id1_d[:])
        wp_sb = pers.tile([128, 2 * 1024], BF, tag="wp")
        nc.sync.dma_start(wp_sb[:], wp_d[:])

        # ---- persistent intermediates ----
        # Q^T, 4 heads side-by-side per q-tile: [64, qt(16) x h(4) x 128]
        q4t = pers.tile([64, NTT * 512], BF, tag="q4t")
        kt_sb = pers.tile([64, T], BF, tag="kt")      # K^T (un-normalized)
        vn_sb = pers.tile([128, NTT * 65], BF, tag="vn")  # V natural + ones col
        # y^T: [128 (2 heads stacked), p(2) x T]
        yt_sb = pers.tile([128, 2 * T], BF, tag="yt")
        rk_sb = pers.tile([128, NTT], F32, tag="rk")  # K rms recip, natural
        g_sb = pers.tile([128, NTT], F32, tag="g")    # sigmoid gates, natural

        nc.vector.memset(vn_sb[:], 1.0)      # ones columns (col 64 of each group)
        biasq_sb = pers.tile([2, 1], F32, tag="biasq")
        nc.vector.memset(biasq_sb[:], 64e-6)
        biask_sb = pers.tile([1, 1], F32, tag="biask")
        nc.vector.memset(biask_sb[:], 1e-6)

        # ---- gates for all t-tiles (sigmoid via exp + reciprocal) ----
        gps = pbS.tile([128, NTT], F32, tag="s", name="gps")
        for tt in range(NTT):
            nc.tensor.matmul(gps[:, tt:tt + 1],
                             xt_sb[0][0:32, tt * 128:(tt + 1) * 128],
                             wg_sb[:], start=True, stop=True)
        eg = sm.tile([128, NTT], F32, tag="u", name="eg")
        nc.scalar.activation(eg[:], gps[:], AF.Exp, scale=-1.0)
        eg1 = sm.tile([128, NTT], F32, tag="u", name="eg1")
        nc.vector.tensor_scalar_add(eg1[:], eg[:], 1.0)
        nc.vector.reciprocal(g_sb[:], eg1[:])

        def qkv_matmul(psum, w_sb, col0, ncol, c0):
            for k in range(8):
                nc.tensor.matmul(
                    psum, w_sb[:, k * ncol + col0: k * ncol + col0 + 128],
                    xt_sb[k][:, c0:c0 + TCH],
                    start=(k == 0), stop=(k == 7))

        for ch in range(NCH):
            c0 = ch * TCH
            csl = slice(c0, c0 + TCH)
            # view of this chunk's q4t region: [64, j(4 qtiles), h(4), c(128)]
            q4v = q4t[:, ch * 2048:(ch + 1) * 2048].rearrange(
                "p (j h c) -> p j h c", j=4, h=4, c=128)

            # ---------------- Q pairs ----------------
            for p in range(2):
                psq = pbA.tile([128, TCH], F32, tag="A", name="psq")
                qkv_matmul(psq, wq_sb, p * 128, 256, c0)
                sq = wk.tile([128, TCH], BF, tag="w", name="sq")
                nc.scalar.square(sq[:], psq[:])
                ss = pbS.tile([2, TCH], F32, tag="s", name="ss")
                nc.tensor.matmul(ss[:], selq_sb[:], sq[:], start=True,
                                 stop=True)
                srt = sm.tile([2, TCH], F32, tag="u", name="srtq")
                nc.scalar.activation(srt[:], ss[:], AF.Sqrt, bias=biasq_sb[:],
                                     scale=1.0)
                rcpf = sm.tile([2, TCH], F32, tag="rf", name="rcpf")
                nc.vector.reciprocal_approx_fast(rcpf[:], srt[:])
                rcpb = sm.tile([2, TCH], BF, tag="rc", name="rcpb")
                nc.vector.tensor_copy(rcpb[:], rcpf[:])
                bcps = pbA.tile([128, TCH], F32, tag="A", name="bcps")
                nc.tensor.matmul(bcps[:], sel128_sb[:], rcpb[:], start=True,
                                 stop=True)
                bcs = wk.tile([128, TCH], BF, tag="w", name="bcs")
                nc.vector.tensor_copy(bcs[:], bcps[:])
                pbn = wk.tile([128, TCH], BF, tag="w", name="pbn")
                nc.vector.tensor_mul(pbn[:], psq[:], bcs[:])
                # A rows: [x1*cos | x2*cos'...]: A = pbn * cs1 -> [x1c, x2s]
                # P2 = pbn * cs2 -> [x1s, x2c]
                # Bsh[h1] = x2*sin (from pbn[h2]*cs1[h2], inputs aligned)
                # Bsh[h2] = x1*sin (from pbn[h1]*cs2[h1], inputs aligned)
                A = wk.tile([128, TCH], BF, tag="w", name="ropeA")
                P2 = wk.tile([128, TCH], BF, tag="w", name="ropeP2")
                Bs = wk.tile([128, TCH], BF, tag="w", name="ropeBs")
                nc.vector.tensor_mul(A[:], pbn[:], cs1_sb[:, csl])
                nc.vector.tensor_mul(P2[:], pbn[:], cs2_sb[:, csl])
                for i in range(2):
                    h = 2 * p + i
                    hh = 64 * i
                    h1 = slice(hh, hh + 32)
                    h2 = slice(hh + 32, hh + 64)
                    nc.vector.tensor_mul(Bs[h1], pbn[h2], cs1_sb[h2, csl])
                    nc.vector.tensor_mul(Bs[h2], pbn[h1], cs2_sb[h1, csl])
                    # out[:32] = x1c - x2s ; out[32:64] = x1s + x2c
                    nc.vector.tensor_sub(q4v[0:32, :, h, :], A[h1], Bs[h1])
                    nc.vector.tensor_add(q4v[32:64, :, h, :], Bs[h2], P2[h2])

            # ---------------- K | V^T ----------------
            pskv = pbA.tile([128, TCH], F32, tag="A", name="pskv")
            qkv_matmul(pskv, wkv_sb, 0, 128, c0)
            sqk = wk.tile([64, TCH], BF, tag="w", name="sqk")
            nc.scalar.square(sqk[:], pskv[0:64])
            ssk = pbS.tile([1, TCH], F32, tag="s", name="ssk")
            nc.tensor.matmul(ssk[:], on64_sb[:], sqk[:], start=True, stop=True)
            srtk = sm.tile([1, TCH], F32, tag="u", name="srtk")
            nc.scalar.activation(srtk[:], ssk[:], AF.Sqrt, bias=biask_sb[:],
                                 scale=1.0 / 64)
            rkf = sm.tile([1, TCH], F32, tag="rf", name="rkf")
            nc.vector.reciprocal_approx_fast(rkf[:], srtk[:])
            rkb = sm.tile([1, TCH], BF, tag="rc", name="rkb")
            nc.vector.tensor_copy(rkb[:], rkf[:])
            for j in range(4):
                tt = ch * 4 + j
                rkp = pbS.tile([128, 1], BF, tag="s", name="rkp")
                nc.tensor.transpose(rkp[:], rkb[:, j * 128:(j + 1) * 128],
                                    id1_sb[:])
                nc.scalar.copy(rk_sb[:, tt:tt + 1], rkp[:])
            pbk = wk.tile([64, TCH], BF, tag="w", name="pbk")
            nc.vector.tensor_copy(pbk[:], pskv[0:64])
            Ak = wk.tile([64, TCH], BF, tag="w", name="ropeAk")
            Pk = wk.tile([64, TCH], BF, tag="w", name="ropePk")
            Bsk = wk.tile([64, TCH], BF, tag="w", name="ropeBsk")
            nc.vector.tensor_mul(Ak[:], pbk[:], cs1_sb[0:64, csl])
            nc.vector.tensor_mul(Pk[:], pbk[:], cs2_sb[0:64, csl])
            nc.vector.tensor_mul(Bsk[0:32], pbk[32:64], cs1_sb[32:64, csl])
            nc.vector.tensor_mul(Bsk[32:64], pbk[0:32], cs2_sb[0:32, csl])
            nc.vector.tensor_sub(kt_sb[0:32, csl], Ak[0:32], Bsk[0:32])
            nc.vector.tensor_add(kt_sb[32:64, csl], Bsk[32:64], Pk[32:64])
            # V natural (+ gate * ve) per t-tile
            vt = wk.tile([64, TCH], BF, tag="w", name="vt")
            nc.scalar.copy(vt[:], pskv[64:128])
            for j in range(4):
                tt = ch * 4 + j
                vtp = pbS.tile([128, 64], BF, tag="s", name="vtp")
                nc.tensor.transpose(vtp[:], vt[:, j * 128:(j + 1) * 128],
                                    id_sb[:])
                nc.vector.scalar_tensor_tensor(
                    vn_sb[:, tt * 65: tt * 65 + 64],
                    ve_sb[:, tt * 64:(tt + 1) * 64], g_sb[:, tt:tt + 1],
                    vtp[:], op0=OP.mult, op1=OP.add)

        # ============ attention + projection (all q-tiles) ============
        if True:
            for qt in range(NTT):
                lo = max(0, qt - 4)
                q_ap = q4t[:, qt * 512:(qt + 1) * 512]
                yext = pbY.tile([65, TCH], F32, tag="y", name="yext")
                for kt in range(lo, qt + 1):
                    st = pbA.tile([128, TCH], F32, tag="A", name="st")
                    nc.tensor.matmul(st[:],
                                     kt_sb[:, kt * 128:(kt + 1) * 128], q_ap,
                                     start=True, stop=True)
                    pt = ptp.tile([128, TCH], BF, tag="pt", name="pt")
                    nc.scalar.activation(pt[:], st[:], AF.Exp,
                                         scale=rk_sb[:, kt:kt + 1])
                    if kt == qt:
                        nc.vector.tensor_mul(pt[:], pt[:], mskc_sb[:])
                    elif kt == qt - 4:
                        nc.vector.tensor_mul(pt[:], pt[:], mskw_sb[:])
                    nc.tensor.matmul(yext[:],
                                     vn_sb[:, kt * 65: kt * 65 + 65], pt[:],
                                     start=(kt == lo), stop=(kt == qt))
                dd = sm.tile([1, TCH], F32, tag="dd", name="dd")
                nc.scalar.copy(dd[:], yext[64:65, :])
                rrf = sm.tile([1, TCH], F32, tag="rf", name="rrf")
                nc.vector.reciprocal_approx_fast(rrf[:], dd[:])
                rrb = sm.tile([1, TCH], BF, tag="rc", name="rrb")
                nc.vector.tensor_copy(rrb[:], rrf[:])
                bcq = pbA.tile([64, TCH], F32, tag="A", name="bcq")
                nc.tensor.matmul(bcq[:], on1x_sb[:], rrb[:], start=True,
                                 stop=True)
                bca = ow.tile([64, TCH], BF, tag="bca", name="bca")
                nc.vector.tensor_copy(bca[:], bcq[:])
                for h in range(4):
                    p, hh = h // 2, (h % 2) * 64
                    nc.vector.tensor_mul(
                        yt_sb[hh:hh + 64, p * T + qt * 128: p * T + (qt + 1) * 128],
                        yext[0:64, h * 128:(h + 1) * 128],
                        bca[:, h * 128:(h + 1) * 128])
                # output projection for this t-tile
                for cc in range(2):
                    ops = pbA.tile([128, TCH], F32, tag="A", name="ops")
                    for p in range(2):
                        nc.tensor.matmul(
                            ops[:], yt_sb[:, p * T + qt * 128: p * T + (qt + 1) * 128],
                            wp_sb[:, p * 1024 + cc * 512: p * 1024 + cc * 512 + 512],
                            start=(p == 0), stop=(p == 1))
                    o_sb = ow.tile([128, TCH], BF, tag="o", name="osb")
                    if cc == 0:
                        nc.scalar.copy(o_sb[:], ops[:])
                    else:
                        nc.vector.tensor_copy(o_sb[:], ops[:])
                    nc.sync.dma_start(
                        out_d[qt * 128:(qt + 1) * 128, cc * 512:(cc + 1) * 512],
                        o_sb[:])

    nc.compile()
    return nc


def _prep_inputs(x, ve, cos, sin, Wq, Wk, Wv, Wproj, Wgate):
    """Build the 8 per-core input maps (host-side sharding + layout prep)."""
    cosT = np.ascontiguousarray(cos.T).astype(np.float32)   # [32, T]
    sinT = np.ascontiguousarray(sin.T).astype(np.float32)
    cs1 = np.concatenate([cosT, sinT, cosT, sinT], 0).astype(BF16)  # [128, T]
    cs2 = np.concatenate([sinT, cosT, sinT, cosT], 0).astype(BF16)
    triu = np.triu(np.ones((128, 128), np.float32))
    tril = np.tril(np.ones((128, 128), np.float32))
    masks = np.concatenate([np.tile(triu, (1, 4)), np.tile(tril, (1, 4))],
                           1).astype(BF16)                  # [128, 1024]
    ident = np.eye(64, dtype=BF16)
    selq2 = np.zeros((128, 2), np.float32)
    selq2[0:64, 0] = 1.0
    selq2[64:128, 1] = 1.0
    selq2 = selq2.astype(BF16)
    sel128 = np.zeros((2, 128), np.float32)
    sel128[0, 0:64] = 1.0
    sel128[1, 64:128] = 1.0
    sel128 = sel128.astype(BF16)
    ones64 = np.ones((64, 1), BF16)
    ones1x64 = np.ones((1, 64), BF16)
    id1 = np.ones((1, 1), BF16)

    xT = [np.ascontiguousarray(x[b].astype(BF16).T).reshape(8, 128, T)
          for b in range(B)]
    in_maps = []
    for c in range(NCORES):
        b, g = c // 4, c % 4
        wq_g = np.ascontiguousarray(np.transpose(
            Wq[:, g * 256:(g + 1) * 256].reshape(8, 128, 256),
            (1, 0, 2)).reshape(128, 8 * 256)).astype(BF16)
        wkv_g = np.ascontiguousarray(np.transpose(np.concatenate(
            [Wk[:, g * 64:(g + 1) * 64], Wv[:, g * 64:(g + 1) * 64]],
            1).reshape(8, 128, 128), (1, 0, 2)).reshape(128, 8 * 128)
        ).astype(BF16)
        wg_g = np.ascontiguousarray(Wgate[:, g:g + 1]).astype(BF16)
        wp_g = np.ascontiguousarray(np.transpose(
            Wproj[g * 256:(g + 1) * 256, :].reshape(2, 128, 1024),
            (1, 0, 2)).reshape(128, 2 * 1024)).astype(BF16)
        ve_g = np.ascontiguousarray(np.transpose(
            (2.0 * ve[b, :, g * 64:(g + 1) * 64]).reshape(16, 128, 64),
            (1, 0, 2)).reshape(128, 16 * 64)).astype(BF16)
        in_maps.append({
            "xt": xT[b], "wq": wq_g, "wkv": wkv_g, "wg": wg_g, "wp": wp_g,
            "cs1": cs1, "cs2": cs2, "ve2": ve_g, "masks": masks,
            "ident": ident, "selq2": selq2, "sel128": sel128,
            "ones64": ones64, "ones1x64": ones1x64, "id1": id1,
        })
    return in_maps


def _run(inputs, trace=False, tmpdir=None):
    if "nc" not in _cache:
        _cache["nc"] = _build()
    nc = _cache["nc"]
    in_maps = _prep_inputs(**inputs)
    res = run_bass_kernel_spmd(nc, in_maps, list(range(NCORES)), trace=trace,
                               tmpdir=tmpdir)
    out = np.zeros((B, T, N_EMBD), np.float32)
    for c in range(NCORES):
        out[c // 4] += np.asarray(res.results[c]["out"]).astype(np.float32)
    return out, res


def kernel(**inputs):
    out, _ = _run(inputs)
    return out
